# revision 5
# baseline (speedup 1.0000x reference)
"""Trainium2 Bass kernel for nn_Blur: 5x5 depthwise Gaussian-like blur.

Strategy
--------
out[b,c,h,w] = sum_{dy,dx in [-2,2]} w2d[dy+2,dx+2] * x[b,c,h+dy,w+dx]  (zero pad)

Data-parallel over batch: 8 cores x 4 images each.

Per core, for each image we put H=128 on the SBUF partition dim and
(channel-chunk, padded-W) on the free dim.  For a fixed column offset dx the
H-convolution is a banded 128x128 matrix A_dx applied on the partition dim,
which TensorE can do as a matmul; the W shift by dx is just a free-dim offset
into a zero-padded (W+4)-wide channel block.  The full conv is then

    out = sum_{dx=-2..2} A_dx @ X_shifted(dx)

i.e. 5 accumulating matmuls per PSUM tile.  Matmuls run in float32r (full-rate
fp32 mode on the PE).  PSUM is drained to SBUF by DVE/ACT copies and DMA'd out.
"""

import numpy as np

import concourse.bass as bass
import concourse.bacc as bacc
import concourse.mybir as mybir
import concourse.tile as tile
from concourse.bass_utils import run_bass_kernel_spmd

N_CORES = 8
B, C, H, W = 32, 128, 128, 128
B_LOC = B // N_CORES  # 4 images per core
KS = 5
PAD = 2
WP = W + 2 * PAD  # 132 padded width per channel block
CH = 32  # channels per chunk (DMA granularity: 2 MiB)
NCHUNK = C // CH
GRP = 4  # channels per matmul (N = GRP*W = 512 = one PSUM bank of fp32)
PSUM_CH = 16  # channels per PSUM tile (4 banks)

_prog_cache = {}


def _build_program():
    nc = bacc.Bacc("TRN2", target_bir_lowering=False, debug=False)
    f32 = mybir.dt.float32
    f32r = mybir.dt.float32r

    x_d = nc.dram_tensor("x", [B_LOC, C, H, W], f32r, kind="ExternalInput")
    w_d = nc.dram_tensor("w", [KS, H, H], f32r, kind="ExternalInput")
    y_d = nc.dram_tensor("y", [B_LOC, C, H, W], f32, kind="ExternalOutput")

    # View DRAM with H first so H maps to the SBUF partition dim.
    x_r = x_d.ap().rearrange("b c h w -> b h c w")
    y_r = y_d.ap().rearrange("b c h w -> b h c w")

    with tile.TileContext(nc) as tc:
        with (
            tc.tile_pool(name="wpool", bufs=1) as wpool,
            tc.tile_pool(name="xpool", bufs=3) as xpool,
            tc.tile_pool(name="opool", bufs=3) as opool,
            tc.tile_pool(name="psum", bufs=2, space="PSUM") as ppool,
        ):
            w_t = wpool.tile([H, KS, H], f32r)
            for t in range(KS):
                nc.sync.dma_start(w_t[:, t, :], w_d.ap()[t])

            # PSUM tile layout is [H, W, PSUM_CH] (w-major, channel innermost)
            # so the fp32r-matmul ISA rules (innermost dst count even, outer
            # steps even) hold for any w-window.  Each matmul fills one PSUM
            # bank = 32 w-positions x 16 channels (N=512).  Tap order: dx=0
            # first (full window, start=True), shifted taps accumulate into
            # w-trimmed sub-windows at the image edge banks (zero padding
            # semantics fall out of PSUM has_written bits).
            taps = [0, -2, -1, 1, 2]
            WB = 32  # w-positions per PSUM bank

            for b in range(B_LOC):
                for cc in range(NCHUNK):
                    c0 = cc * CH
                    xt = xpool.tile([H, CH, W], f32r)
                    nc.sync.dma_start(xt[:], x_r[b, :, c0 : c0 + CH, :])
                    ot = opool.tile([H, CH, W], f32)
                    for half in range(CH // PSUM_CH):
                        h0 = half * PSUM_CH
                        ps = ppool.tile([H, W, PSUM_CH], f32)
                        for g in range(W // WB):
                            for ti, dx in enumerate(taps):
                                t = dx + PAD
                                wa = max(g * WB, -dx)
                                wb = min((g + 1) * WB, W - dx)
                                nc.tensor.matmul(
                                    ps[:, wa:wb, :],
                                    w_t[:, t, :],
                                    xt[
                                        :,
                                        h0 : h0 + PSUM_CH,
                                        wa + dx : wb + dx,
                                    ].rearrange("p c w -> p w c"),
                                    start=(ti == 0),
                                    stop=(ti == KS - 1),
                                )
                        # Drain PSUM -> SBUF (un-transposing w/c), split
                        # across DVE and ACT.
                        hh = PSUM_CH // 2
                        nc.vector.tensor_copy(
                            ot[:, h0 : h0 + hh, :],
                            ps[:, :, :hh].rearrange("p w c -> p c w"),
                        )
                        nc.scalar.copy(
                            ot[:, h0 + hh : h0 + PSUM_CH, :],
                            ps[:, :, hh:].rearrange("p w c -> p c w"),
                        )
                    nc.scalar.dma_start(y_r[b, :, c0 : c0 + CH, :], ot[:])

    nc.compile()
    return nc


def _blur_matrices(log_lengthscale: np.ndarray) -> np.ndarray:
    """Host-side: 5x5 kernel (fp32, mirroring the reference numerics) spread
    into 5 banded [H,H] lhsT matrices, one per column offset dx.

    lhsT[t][h_in, h_out] = w2d[h_in - h_out + 2, t]  for |h_in - h_out| <= 2.
    """
    ls = np.exp(np.float32(log_lengthscale))
    coords = (np.arange(KS, dtype=np.float32) - KS // 2).astype(np.float32)
    d2 = (coords[:, None] ** 2 + coords[None, :] ** 2).astype(np.float32) ** 2
    unscaled = np.exp((-d2 / (np.float32(2.0) * ls)).astype(np.float32))
    w2d = (unscaled / unscaled.sum(dtype=np.float32)).astype(np.float32)

    mats = np.zeros((KS, H, H), dtype=np.float32)
    for t in range(KS):
        for d in range(-PAD, PAD + 1):
            v = w2d[d + PAD, t]
            idx = np.arange(max(0, -d), min(H, H - d))  # h_out range
            mats[t][idx + d, idx] = v
    return mats


def kernel(x: np.ndarray, log_lengthscale: np.ndarray) -> np.ndarray:
    x = np.ascontiguousarray(x, dtype=np.float32)
    mats = _blur_matrices(np.asarray(log_lengthscale, dtype=np.float32))

    if "nc" not in _prog_cache:
        _prog_cache["nc"] = _build_program()
    nc = _prog_cache["nc"]

    in_maps = [
        {"x": x[i * B_LOC : (i + 1) * B_LOC], "w": mats} for i in range(N_CORES)
    ]
    res = run_bass_kernel_spmd(nc, in_maps, list(range(N_CORES)))
    return np.concatenate([res.results[i]["y"] for i in range(N_CORES)], axis=0)


# revision 8
# speedup vs baseline: 7.0425x; 7.0425x over previous
"""Trainium2 Bass kernel for nn_Blur: 5x5 depthwise Gaussian-like blur.

Strategy
--------
out[b,c,h,w] = sum_{dy,dx in [-2,2]} w2d[dy+2,dx+2] * x[b,c,h+dy,w+dx]  (zero pad)

Data-parallel over batch: 8 cores x 4 images each.

Per core, for each image we put H=128 on the SBUF partition dim and
(channel-chunk, padded-W) on the free dim.  For a fixed column offset dx the
H-convolution is a banded 128x128 matrix A_dx applied on the partition dim,
which TensorE can do as a matmul; the W shift by dx is just a free-dim offset
into a zero-padded (W+4)-wide channel block.  The full conv is then

    out = sum_{dx=-2..2} A_dx @ X_shifted(dx)

i.e. 5 accumulating matmuls per PSUM tile.  Matmuls run in float32r (full-rate
fp32 mode on the PE).  PSUM is drained to SBUF by DVE/ACT copies and DMA'd out.
"""

import numpy as np

import concourse.bass as bass
import concourse.bacc as bacc
import concourse.mybir as mybir
import concourse.tile as tile
from concourse.bass_utils import run_bass_kernel_spmd

N_CORES = 8
B, C, H, W = 32, 128, 128, 128
B_LOC = B // N_CORES  # 4 images per core
KS = 5
PAD = 2
WP = W + 2 * PAD  # 132 padded width per channel block
CH = 32  # channels per chunk (DMA granularity: 2 MiB)
NCHUNK = C // CH
GRP = 4  # channels per matmul (N = GRP*W = 512 = one PSUM bank of fp32)
PSUM_CH = 16  # channels per PSUM tile (4 banks)

_prog_cache = {}


def _build_program(loop_n: int = 1):
    """loop_n > 1 wraps the whole body in a hardware loop (benchmarking
    only; repeats the identical computation loop_n times per execution)."""
    nc = bacc.Bacc("TRN2", target_bir_lowering=False, debug=False)
    f32 = mybir.dt.float32
    f32r = mybir.dt.float32r

    x_d = nc.dram_tensor("x", [B_LOC, C, H, W], f32r, kind="ExternalInput")
    w_d = nc.dram_tensor("w", [KS, H, H], f32r, kind="ExternalInput")
    y_d = nc.dram_tensor("y", [B_LOC, C, H, W], f32, kind="ExternalOutput")

    # View DRAM with H first so H maps to the SBUF partition dim.
    x_r = x_d.ap().rearrange("b c h w -> b h c w")
    y_r = y_d.ap().rearrange("b c h w -> b h c w")

    with tile.TileContext(nc) as tc:
        with (
            tc.tile_pool(name="wpool", bufs=1) as wpool,
            tc.tile_pool(name="xpool", bufs=3) as xpool,
            tc.tile_pool(name="opool", bufs=3) as opool,
            tc.tile_pool(name="psum", bufs=2, space="PSUM") as ppool,
        ):
            w_t = wpool.tile([H, KS, H], f32r)
            for t in range(KS):
                nc.sync.dma_start(w_t[:, t, :], w_d.ap()[t])

            # PSUM tile layout is [H, W, PSUM_CH] (w-major, channel innermost)
            # so the fp32r-matmul ISA rules (innermost dst count even, outer
            # steps even) hold for any w-window.  Each matmul fills one PSUM
            # bank = 32 w-positions x 16 channels (N=512).  Tap order: dx=0
            # first (full window, start=True), shifted taps accumulate into
            # w-trimmed sub-windows at the image edge banks (zero padding
            # semantics fall out of PSUM has_written bits).
            taps = [0, -2, -1, 1, 2]
            WB = 32  # w-positions per PSUM bank

            import contextlib

            loop_ctx = (
                tc.For_i(0, loop_n, 1)
                if loop_n > 1
                else contextlib.nullcontext()
            )
            with loop_ctx:
                _emit_body(nc, tc, x_r, y_r, w_t, xpool, opool, ppool, taps, WB)

    nc.compile()
    return nc


def _emit_body(nc, tc, x_r, y_r, w_t, xpool, opool, ppool, taps, WB):
    f32 = mybir.dt.float32
    f32r = mybir.dt.float32r
    if True:
        if True:
            for b in range(B_LOC):
                for cc in range(NCHUNK):
                    c0 = cc * CH
                    xt = xpool.tile([H, CH, W], f32r)
                    nc.sync.dma_start(xt[:], x_r[b, :, c0 : c0 + CH, :])
                    ot = opool.tile([H, CH, W], f32)
                    for half in range(CH // PSUM_CH):
                        h0 = half * PSUM_CH
                        ps = ppool.tile([H, W, PSUM_CH], f32)
                        for g in range(W // WB):
                            for ti, dx in enumerate(taps):
                                t = dx + PAD
                                wa = max(g * WB, -dx)
                                wb = min((g + 1) * WB, W - dx)
                                nc.tensor.matmul(
                                    ps[:, wa:wb, :],
                                    w_t[:, t, :],
                                    xt[
                                        :,
                                        h0 : h0 + PSUM_CH,
                                        wa + dx : wb + dx,
                                    ].rearrange("p c w -> p w c"),
                                    start=(ti == 0),
                                    stop=(ti == KS - 1),
                                )
                        # Drain PSUM -> SBUF (un-transposing w/c), split
                        # across DVE and ACT.
                        hh = PSUM_CH // 2
                        nc.vector.tensor_copy(
                            ot[:, h0 : h0 + hh, :],
                            ps[:, :, :hh].rearrange("p w c -> p c w"),
                        )
                        nc.scalar.copy(
                            ot[:, h0 + hh : h0 + PSUM_CH, :],
                            ps[:, :, hh:].rearrange("p w c -> p c w"),
                        )
                    nc.scalar.dma_start(y_r[b, :, c0 : c0 + CH, :], ot[:])


def _blur_matrices(log_lengthscale: np.ndarray) -> np.ndarray:
    """Host-side: 5x5 kernel (fp32, mirroring the reference numerics) spread
    into 5 banded [H,H] lhsT matrices, one per column offset dx.

    lhsT[t][h_in, h_out] = w2d[h_in - h_out + 2, t]  for |h_in - h_out| <= 2.
    """
    ls = np.exp(np.float32(log_lengthscale))
    coords = (np.arange(KS, dtype=np.float32) - KS // 2).astype(np.float32)
    d2 = (coords[:, None] ** 2 + coords[None, :] ** 2).astype(np.float32) ** 2
    unscaled = np.exp((-d2 / (np.float32(2.0) * ls)).astype(np.float32))
    w2d = (unscaled / unscaled.sum(dtype=np.float32)).astype(np.float32)

    mats = np.zeros((KS, H, H), dtype=np.float32)
    for t in range(KS):
        for d in range(-PAD, PAD + 1):
            v = w2d[d + PAD, t]
            idx = np.arange(max(0, -d), min(H, H - d))  # h_out range
            mats[t][idx + d, idx] = v
    return mats


def kernel(x: np.ndarray, log_lengthscale: np.ndarray) -> np.ndarray:
    x = np.ascontiguousarray(x, dtype=np.float32)
    mats = _blur_matrices(np.asarray(log_lengthscale, dtype=np.float32))

    if "nc" not in _prog_cache:
        _prog_cache["nc"] = _build_program()
    nc = _prog_cache["nc"]

    in_maps = [
        {"x": x[i * B_LOC : (i + 1) * B_LOC], "w": mats} for i in range(N_CORES)
    ]
    res = run_bass_kernel_spmd(nc, in_maps, list(range(N_CORES)))
    return np.concatenate([res.results[i]["y"] for i in range(N_CORES)], axis=0)


# revision 22
# speedup vs baseline: 10.2341x; 1.4532x over previous
"""Trainium2 Bass kernel for nn_Blur: 5x5 depthwise Gaussian-like blur.

Strategy
--------
out[b,c,h,w] = sum_{dy,dx in [-2,2]} w2d[dy+2,dx+2] * x[b,c,h+dy,w+dx]  (zero pad)

Data-parallel over batch: 8 cores x 4 images each.

Per core, for each image we put H=128 on the SBUF partition dim and
(channel-chunk, W) on the free dim.  For a fixed column offset dx the
H-convolution is a banded 128x128 matrix A_dx applied on the partition dim,
which TensorE does as a matmul; the W shift by dx is a free-dim offset:

    out = sum_{dx=-2..2} A_dx @ X_shifted(dx)

i.e. 5 accumulating matmuls per PSUM bank, in float32r (full-rate fp32 PE
mode; ~1e-4 rel err).  W-boundary columns are handled by even-aligned
sub-windows (fp32r ISA alignment rules) plus two tiny edge-fix matmuls per
chunk.  PSUM drains to SBUF via DVE/ACT copies; in/out DMAs alternate
between the two HWDGE rings per chunk.  Measured ~198 us/core vs the
~187 us HBM roofline (64 MiB traffic at ~358 GB/s).
"""

import numpy as np

import concourse.bass as bass
import concourse.bacc as bacc
import concourse.mybir as mybir
import concourse.tile as tile
from concourse.bass_utils import run_bass_kernel_spmd

N_CORES = 8
B, C, H, W = 32, 128, 128, 128
B_LOC = B // N_CORES  # 4 images per core
KS = 5
PAD = 2
WP = W + 2 * PAD  # 132 padded width per channel block
CH = 32  # channels per chunk (DMA granularity)
NCHUNK = C // CH
GRP = 4  # channels per matmul (N = GRP*W = 512 = one PSUM bank of fp32)
PSUM_CH = 8  # channels per PSUM tile (2 banks)
XBUFS = 4
OBUFS = 4
PBUFS = 3

_prog_cache = {}


def _build_program(loop_n: int = 1, variant: str = "full"):
    """loop_n > 1 wraps the whole body in a hardware loop (benchmarking
    only; repeats the identical computation loop_n times per execution).
    variant: "full" | "dma" (DMA in/out only) | "compute" (no big DMAs)."""
    nc = bacc.Bacc("TRN2", target_bir_lowering=False, debug=False)
    f32 = mybir.dt.float32
    f32r = mybir.dt.float32r

    x_d = nc.dram_tensor("x", [B_LOC, C, H, W], f32r, kind="ExternalInput")
    w_d = nc.dram_tensor("w", [KS, H, H], f32r, kind="ExternalInput")
    y_d = nc.dram_tensor("y", [B_LOC, C, H, W], f32, kind="ExternalOutput")

    # View DRAM with H first so H maps to the SBUF partition dim.
    x_r = x_d.ap().rearrange("b c h w -> b h c w")
    y_r = y_d.ap().rearrange("b c h w -> b h c w")

    with tile.TileContext(nc) as tc:
        with (
            tc.tile_pool(name="wpool", bufs=1) as wpool,
            tc.tile_pool(name="xpool", bufs=XBUFS) as xpool,
            tc.tile_pool(name="opool", bufs=OBUFS) as opool,
            tc.tile_pool(name="psum", bufs=PBUFS, space="PSUM") as ppool,
        ):
            w_t = wpool.tile([H, KS, H], f32r)
            for t in range(KS):
                nc.sync.dma_start(w_t[:, t, :], w_d.ap()[t])

            # c-major PSUM layout [H, PSUM_CH, W]: contiguous moving-operand
            # streaming.  fp32r-matmul ISA rules (innermost count even, dst
            # byte offset % 8 == 0) are satisfied by asymmetric even windows:
            # w in [2 if dx<0 else 0, 126 if dx>0 else 128).  That covers the
            # full valid range for dx in {0,+-2} and misses exactly one valid
            # column for dx=+1 (w=126) and dx=-1 (w=1); those two columns are
            # computed via a tiny auxiliary PSUM tile (2 N=CH matmuls per
            # chunk) and added into the output tile on DVE.  Tap dx=0 goes
            # first (full bank window, start=True); shifted taps accumulate
            # into sub-windows (zero-padding semantics fall out of PSUM
            # has_written bits).
            taps = [0, -2, -1, 1, 2]
            WB = 32  # unused (kept for signature stability)

            import contextlib

            loop_ctx = (
                tc.For_i(0, loop_n, 1)
                if loop_n > 1
                else contextlib.nullcontext()
            )
            xt0 = None
            if variant == "compute":
                xt0 = xpool.tile([H, CH, W], f32r, name="xt0")
                nc.sync.dma_start(xt0[:], x_r[0, :, 0:CH, :])

            with loop_ctx:
                _emit_body(
                    nc, tc, x_r, y_r, w_t, xpool, opool, ppool, taps, WB,
                    variant, xt0,
                )

    nc.compile()
    return nc


def _emit_body(
    nc, tc, x_r, y_r, w_t, xpool, opool, ppool, taps, WB, variant, xt0
):
    f32 = mybir.dt.float32
    f32r = mybir.dt.float32r
    for b in range(B_LOC):
        for cc in range(NCHUNK):
            c0 = cc * CH
            if variant == "dma":
                xt = xpool.tile([H, CH, W], f32r)
                nc.sync.dma_start(xt[:], x_r[b, :, c0 : c0 + CH, :])
                nc.scalar.dma_start(
                    y_r[b, :, c0 : c0 + CH, :], xt[:].bitcast(f32)
                )
                continue
            k = b * NCHUNK + cc
            eng_in = nc.sync if k % 2 == 0 else nc.scalar
            eng_out = nc.scalar if k % 2 == 0 else nc.sync
            if variant == "compute":
                xt = xt0
            else:
                xt = xpool.tile([H, CH, W], f32r)
                eng_in.dma_start(xt[:], x_r[b, :, c0 : c0 + CH, :])
            ot = opool.tile([H, CH, W], f32)
            for half in range(CH // PSUM_CH):
                h0 = half * PSUM_CH
                ps = ppool.tile([H, PSUM_CH, W], f32, tag="ps")
                for g in range(PSUM_CH // GRP):
                    cg = h0 + g * GRP
                    for ti, dx in enumerate(taps):
                        t = dx + PAD
                        wa = 2 if dx < 0 else 0
                        wb = 126 if dx > 0 else W
                        nc.tensor.matmul(
                            ps[:, g * GRP : (g + 1) * GRP, wa:wb],
                            w_t[:, t, :],
                            xt[:, cg : cg + GRP, wa + dx : wb + dx],
                            start=(ti == 0),
                            stop=(ti == KS - 1),
                        )
                # Drain PSUM -> SBUF, split across DVE and ACT.
                hh = PSUM_CH // 2
                nc.vector.tensor_copy(ot[:, h0 : h0 + hh, :], ps[:, :hh, :])
                nc.scalar.copy(
                    ot[:, h0 + hh : h0 + PSUM_CH, :], ps[:, hh:, :]
                )
            # The two missing edge columns: out[:, c, 126] += A_{+1} x[:, c, 127]
            # and out[:, c, 1] += A_{-1} x[:, c, 0], batched over the chunk.
            pe = ppool.tile([H, 2, CH], f32, tag="pe", bufs=2)
            nc.tensor.matmul(
                pe[:, 0, :], w_t[:, PAD + 1, :], xt[:, :, 127],
                start=True, stop=False,
            )
            nc.tensor.matmul(
                pe[:, 1, :], w_t[:, PAD - 1, :], xt[:, :, 0],
                start=False, stop=True,
            )
            nc.vector.tensor_add(ot[:, :, 126], ot[:, :, 126], pe[:, 0, :])
            nc.vector.tensor_add(ot[:, :, 1], ot[:, :, 1], pe[:, 1, :])
            if variant == "full":
                eng_out.dma_start(y_r[b, :, c0 : c0 + CH, :], ot[:])


def _blur_matrices(log_lengthscale: np.ndarray) -> np.ndarray:
    """Host-side: 5x5 kernel (fp32, mirroring the reference numerics) spread
    into 5 banded [H,H] lhsT matrices, one per column offset dx.

    lhsT[t][h_in, h_out] = w2d[h_in - h_out + 2, t]  for |h_in - h_out| <= 2.
    """
    ls = np.exp(np.float32(log_lengthscale))
    coords = (np.arange(KS, dtype=np.float32) - KS // 2).astype(np.float32)
    d2 = (coords[:, None] ** 2 + coords[None, :] ** 2).astype(np.float32) ** 2
    unscaled = np.exp((-d2 / (np.float32(2.0) * ls)).astype(np.float32))
    w2d = (unscaled / unscaled.sum(dtype=np.float32)).astype(np.float32)

    mats = np.zeros((KS, H, H), dtype=np.float32)
    for t in range(KS):
        for d in range(-PAD, PAD + 1):
            v = w2d[d + PAD, t]
            idx = np.arange(max(0, -d), min(H, H - d))  # h_out range
            mats[t][idx + d, idx] = v
    return mats


def kernel(x: np.ndarray, log_lengthscale: np.ndarray) -> np.ndarray:
    x = np.ascontiguousarray(x, dtype=np.float32)
    mats = _blur_matrices(np.asarray(log_lengthscale, dtype=np.float32))

    if "nc" not in _prog_cache:
        _prog_cache["nc"] = _build_program()
    nc = _prog_cache["nc"]

    in_maps = [
        {"x": x[i * B_LOC : (i + 1) * B_LOC], "w": mats} for i in range(N_CORES)
    ]
    res = run_bass_kernel_spmd(nc, in_maps, list(range(N_CORES)))
    return np.concatenate([res.results[i]["y"] for i in range(N_CORES)], axis=0)
